# revision 20
# baseline (speedup 1.0000x reference)
"""Trainium2 Bass kernel for the consistency-loss problem.

loss = -mean_b( table[argmax_c pred1[b,c]] . log_softmax(pred2[b]) )

Algebra used on-device (per batch row b, with c* = argmax of pred1 row):
    loss_b = lse_b * s[c*] - table[c*] . pred2[b]
where lse_b = log(sum_j exp(pred2[b,j])) and s[c] = sum_j table[c,j].

The expensive dot term summed over the batch factorizes through a matmul in
the natural (row-major) layout:
    sum_b table[c*_b] . pred2[b] = sum_{c,j} table[c,j] * G[c,j],
    G = onehot(c*)^T @ pred2          (contraction over batch rows)
so the PE accumulates G in PSUM across row-tiles (f32r) with no transposes
of the big [B, 1000] tensor.

The kernel is HBM-bound (~358 GB/s per core): 36.4 MB of inputs stream in
~102 us, so the design minimizes everything off the stream:
 - pred1 + table ride the SWDGE (gpsimd) queue starting ~2.5 us, before the
   sync-ring HWDGE descriptors are even generated; pred2 owns the sync ring.
 - row layout r = p*64 + i*4 + k gives 16 KB contiguous runs per pred2 tile
   and 12.8 KB runs for the two pred1 half-loads.
 - per-row exp sums: ACT Exp (some with fused accumulation), DVE row-sum
   reduces; the one-hot compare runs as four broadcasted DVE ops.
 - no device-side log: the [128, 64] exp row-sums are shipped to the host
   (32 KB), which applies log in f64 — this removes the natural_log ACT
   table switch (~2.7 us) and the final reduction chain from the tail.
 - G folds against the table on the DVE: tiles 0..12 fold mid-stream
   (hidden), tiles 13..15 fold in the ~2.4 us tail.

The harness table is row-stochastic (uniform row sums), so the fast program
skips the per-row s[c*] gather; the host scales sum(log se) by s0.  A
general program (per-row s via the one-hot) remains as a fallback.

Sharding: data-parallel over B across 8 NeuronCores; the [100,1000] table
is replicated; the host combines per-core partial outputs.
"""

import sys
from contextlib import ExitStack

import numpy as np

for _p in ("/opt/trn_rl_repo", "/root/.axon_site/_ro/trn_rl_repo"):
    if _p not in sys.path:
        sys.path.append(_p)

import concourse.bass as bass
import concourse.tile as tile
from concourse import bacc, mybir
from concourse.bass_utils import run_bass_kernel_spmd

B, C1, C2 = 65536, 100, 1000
NCORES = 8
BC = B // NCORES            # rows per core (8192)
P = 128                     # partitions
KS = 4                      # sub-rows per partition per tile
NT = BC // (P * KS)         # tiles per core (16)
NSEG = BC // P              # per-row segments per core (64) == NT*KS
NQ = 4                      # one-hot quarters (DVE op granularity)
JQ = NSEG // NQ             # segments per quarter (16)
F32 = mybir.dt.float32
F32R = mybir.dt.float32r
X = mybir.AxisListType.X
ALU = mybir.AluOpType
ACTF = mybir.ActivationFunctionType

# PSUM matmul chunking of the C2 free dim (PSUM bank holds 512 f32).
CHUNKS = [(0, 512), (512, C2)]
GA_LAST = NT - 4            # G splits: tiles 0..12 (folded early) / 13..15
# Exp row-sums fused on the ACT accumulator (mid-stream, where ACT idles
# waiting on DMA anyway); the rest reduce on the DVE.  Tail tiles stay on
# the DVE so the last segments' sums come off the parallel pipe; seg 63
# uses the ACT accumulator so the DVE is free for the G_b fold.
ACT_ACCUM_SEGS = frozenset(s for s in range(4, 56, 3)) | {NSEG - 1}


def _build_program(general: bool) -> bass.Bass:
    nc = bacc.Bacc("TRN2", target_bir_lowering=False, debug=False,
                   num_devices=NCORES)
    p1 = nc.dram_tensor("p1", [BC, C1], F32, kind="ExternalInput").ap()
    p2 = nc.dram_tensor("p2", [BC, C2], F32, kind="ExternalInput").ap()
    tbl = nc.dram_tensor("tbl", [C1, C2], F32, kind="ExternalInput").ap()
    sbc = None
    if general:
        # table row-sums broadcast over partitions (host constant)
        sbc = nc.dram_tensor("sbc", [P, C1], F32, kind="ExternalInput").ap()
    se_out = nc.dram_tensor("se", [P, NSEG], F32, kind="ExternalOutput").ap()
    rd_out = nc.dram_tensor("rd", [C1, 2], F32, kind="ExternalOutput").ap()
    sel_out = None
    if general:
        sel_out = nc.dram_tensor("sel", [P, NSEG], F32,
                                 kind="ExternalOutput").ap()

    with tile.TileContext(nc) as tc:
        with ExitStack() as ctx:
            _kernel_body(ctx, tc, p1, p2, tbl, sbc, se_out, rd_out, sel_out,
                         general)
    nc.compile()
    return nc


def _kernel_body(ctx: ExitStack, tc, p1, p2, tbl, sbc, se_out, rd_out,
                 sel_out, general):
    nc = tc.nc
    consts = ctx.enter_context(tc.tile_pool(name="consts", bufs=1))
    p2pool = ctx.enter_context(tc.tile_pool(name="p2", bufs=6))
    expp = ctx.enter_context(tc.tile_pool(name="expp", bufs=6))
    psum = ctx.enter_context(tc.tile_pool(name="psum", bufs=1, space="PSUM"))

    # row (p*64 + i*4 + k)  <->  tile i, partition p, sub-row k
    p2t = p2.rearrange("(p i k) c -> i p (k c)", p=P, i=NT, k=KS)
    # pred1 halves: per-partition 12.8KB contiguous runs
    p1h = p1.rearrange("(p h j) c -> h p (j c)", p=P, h=2, j=NSEG // 2)

    p1big = consts.tile([P, NSEG * C1], F32)
    oh_all = consts.tile([P, NSEG * C1], F32R)
    tbl_sb = consts.tile([C1, C2], F32)
    rmax = consts.tile([P, NSEG], F32)
    se_all = consts.tile([P, NSEG], F32)
    rowdots = consts.tile([C1, 2], F32)

    if general:
        sbc_sb = consts.tile([P, C1], F32)
        ss_scratch = consts.tile([P, JQ * C1], F32)
        ss3 = ss_scratch[:].rearrange("p (j c) -> p j c", j=JQ)
        sbc3 = sbc_sb[:].unsqueeze(1).broadcast_to((P, JQ, C1))
        sel_s_all = consts.tile([P, NSEG], F32)

    G_a = psum.tile([C1, C2], F32)         # onehot^T @ pred2, tiles 0..12
    G_b = psum.tile([C1, C2], F32)         # tiles 13..15

    p1big3 = p1big[:].rearrange("p (j c) -> p j c", j=NSEG)
    oh3 = oh_all[:].rearrange("p (j c) -> p j c", j=NSEG)

    def onehot_quarter(q):
        js = slice(q * JQ, (q + 1) * JQ)
        nc.vector.reduce_max(rmax[:, js], p1big3[:, js, :], axis=X)
        rmx3 = rmax[:, js].unsqueeze(2).broadcast_to((P, JQ, C1))
        nc.vector.tensor_tensor(oh3[:, js, :], p1big3[:, js, :], rmx3,
                                op=ALU.is_ge)
        if general:
            nc.vector.tensor_tensor(ss3[:], oh3[:, js, :].bitcast(F32), sbc3,
                                    op=ALU.mult)
            nc.vector.reduce_sum(sel_s_all[:, js], ss3[:], axis=X)

    t2_tiles = []

    def load_tile(i):
        t2 = p2pool.tile([P, KS * C2], F32R, tag="p2")
        if i == 0 or i == NT - 1:
            # k-split: pipeline head starts early / tail lands continuously
            for k in range(KS):
                nc.sync.dma_start(t2[:, bass.ts(k, C2)],
                                  p2t[i][:, bass.ts(k, C2)].bitcast(F32R))
        elif i >= NT - 3:
            # pair-split taper for the tiles feeding the serialized ACT tail
            for k in range(0, KS, 2):
                nc.sync.dma_start(t2[:, k * C2:(k + 2) * C2],
                                  p2t[i][:, k * C2:(k + 2) * C2].bitcast(F32R))
        else:
            nc.sync.dma_start(t2[:], p2t[i].bitcast(F32R))
        t2_tiles.append(t2)

    def consume_tile(i):
        t2 = t2_tiles[i]
        for k in range(KS):
            seg = i * KS + k
            se_col = se_all[:, seg:seg + 1]
            et = expp.tile([P, C2], F32, tag="exp1")
            if seg in ACT_ACCUM_SEGS:
                nc.scalar.activation(et[:], t2[:, bass.ts(k, C2)].bitcast(F32),
                                     ACTF.Exp, accum_out=se_col)
            else:
                nc.scalar.activation(et[:], t2[:, bass.ts(k, C2)].bitcast(F32),
                                     ACTF.Exp)
                nc.vector.reduce_sum(se_col, et[:], axis=X)
        G = G_a if i <= GA_LAST else G_b
        for k in range(KS):
            seg = i * KS + k
            for lo, hi in CHUNKS:
                nc.tensor.matmul(G[:, lo:hi], oh_all[:, bass.ts(seg, C1)],
                                 t2[:, k * C2 + lo:k * C2 + hi],
                                 start=(k == 0 and i in (0, GA_LAST + 1)),
                                 stop=(k == KS - 1 and i in (GA_LAST, NT - 1)))

    # pred1 + table ride the SWDGE queue in parallel with the sync-ring
    # pred2 stream: the two descriptor paths overlap, keeping HBM saturated
    # without serializing pred2 tiles behind pred1 in one FIFO.
    nc.gpsimd.dma_start(p1big[:, 0:NSEG * C1 // 2], p1h[0])
    nc.gpsimd.dma_start(p1big[:, NSEG * C1 // 2:], p1h[1])
    nc.gpsimd.dma_start(tbl_sb[:], tbl[:, :])
    if general:
        nc.gpsimd.dma_start(sbc_sb[:], sbc[:, :])
    for i in range(NT):
        load_tile(i)

    gt_scratch = consts.tile([C1, C2], F32)
    for i in range(NT):
        if i % 4 == 0:
            onehot_quarter(i // 4)
        consume_tile(i)
        if i == GA_LAST + 1:
            # G_a is complete; fold it against the table mid-stream (fused
            # multiply + negated row-reduce on the DVE, fully hidden) and
            # ship the result while pred2 still streams.
            nc.vector.tensor_mul(gt_scratch[:], G_a[:], tbl_sb[:])
            nc.vector.tensor_reduce(rowdots[:, 0:1], gt_scratch[:], axis=X,
                                    op=ALU.add, negate=True)

    # Tail: G_b fold on the DVE, then the two small result DMAs.
    nc.vector.tensor_mul(gt_scratch[:], G_b[:], tbl_sb[:])
    nc.vector.tensor_reduce(rowdots[:, 1:2], gt_scratch[:], axis=X,
                            op=ALU.add, negate=True)
    nc.sync.dma_start(se_out[:, :], se_all[:])
    if general:
        nc.sync.dma_start(sel_out[:, :], sel_s_all[:])
    nc.sync.dma_start(rd_out[:, :], rowdots[:])


_PROGRAM_CACHE: dict = {}


def _program(general: bool = False) -> bass.Bass:
    key = "general" if general else "fast"
    if key not in _PROGRAM_CACHE:
        _PROGRAM_CACHE[key] = _build_program(general)
    return _PROGRAM_CACHE[key]


def _row_sums(table):
    return np.asarray(table, dtype=np.float32).sum(axis=1, dtype=np.float32)


def _is_uniform_s(s):
    return bool(np.all(np.abs(s - s[0]) <= 1e-6 * max(1.0, abs(float(s[0])))))


def _in_maps(pred1_logits, pred2_logits, table, general: bool = False):
    p1 = np.ascontiguousarray(pred1_logits, dtype=np.float32)
    p2 = np.ascontiguousarray(pred2_logits, dtype=np.float32)
    tbl = np.ascontiguousarray(table, dtype=np.float32)
    maps = []
    for k in range(NCORES):
        m = {
            "p1": np.ascontiguousarray(p1[k * BC:(k + 1) * BC]),
            "p2": np.ascontiguousarray(p2[k * BC:(k + 1) * BC]),
            "tbl": tbl,
        }
        if general:
            s = _row_sums(tbl)
            m["sbc"] = np.ascontiguousarray(np.tile(s, (P, 1)))
        maps.append(m)
    return maps


def _combine(result, s0, general):
    lse = np.log(np.asarray(result["se"], dtype=np.float64))
    if general:
        lse = lse * np.asarray(result["sel"], dtype=np.float64)
        lse_term = lse.sum()
    else:
        lse_term = s0 * lse.sum()
    return lse_term + np.asarray(result["rd"], dtype=np.float64).sum()


def run_on_device(pred1_logits, pred2_logits, table, **spmd_kwargs):
    """Compile/run the SPMD program on cores 0-7; returns (loss, results)."""
    s = _row_sums(table)
    general = not _is_uniform_s(s)
    nc = _program(general)
    res = run_bass_kernel_spmd(
        nc, _in_maps(pred1_logits, pred2_logits, table, general),
        core_ids=list(range(NCORES)), **spmd_kwargs)
    s0 = np.float64(s[0])
    partials = [_combine(r, s0, general) for r in res.results]
    loss = np.float32(np.sum(partials, dtype=np.float64) / B)
    return np.asarray(loss), res


def kernel(pred1_logits, pred2_logits, table):
    loss, _ = run_on_device(pred1_logits, pred2_logits, table)
    return loss


# revision 21
# speedup vs baseline: 1.0992x; 1.0992x over previous
"""Trainium2 Bass kernel for the consistency-loss problem.

loss = -mean_b( table[argmax_c pred1[b,c]] . log_softmax(pred2[b]) )

Algebra used on-device (per batch row b, with c* = argmax of pred1 row):
    loss_b = lse_b * s[c*] - table[c*] . pred2[b]
where lse_b = log(sum_j exp(pred2[b,j])) and s[c] = sum_j table[c,j].

The expensive dot term summed over the batch factorizes through a matmul in
the natural (row-major) layout:
    sum_b table[c*_b] . pred2[b] = sum_{c,j} table[c,j] * G[c,j],
    G = onehot(c*)^T @ pred2          (contraction over batch rows)
so the PE accumulates G in PSUM across row-tiles (f32r) with no transposes
of the big [B, 1000] tensor.

The kernel is HBM-bound (~358 GB/s per core): 36.4 MB of inputs stream in
~102 us, so the design minimizes everything off the stream:
 - pred1 + table ride the SWDGE (gpsimd) queue starting ~2.5 us, before the
   sync-ring HWDGE descriptors are even generated; pred2 owns the sync ring.
 - row layout r = p*64 + i*4 + k gives 16 KB contiguous runs per pred2 tile
   and 12.8 KB runs for the two pred1 half-loads.
 - per-row exp sums: ACT Exp (some with fused accumulation), DVE row-sum
   reduces; the one-hot compare runs as four broadcasted DVE ops.
 - no device-side log: the [128, 64] exp row-sums are shipped to the host
   (32 KB), which applies log in f64 — this removes the natural_log ACT
   table switch (~2.7 us) and the final reduction chain from the tail.
 - G folds against the table on the DVE: tiles 0..12 fold mid-stream
   (hidden), tiles 13..15 fold in the ~2.4 us tail.

The harness table is row-stochastic (uniform row sums), so the fast program
skips the per-row s[c*] gather; the host scales sum(log se) by s0.  A
general program (per-row s via the one-hot) remains as a fallback.

Sharding: data-parallel over B across 8 NeuronCores; the [100,1000] table
is replicated; the host combines per-core partial outputs.
"""

import sys
from contextlib import ExitStack

import numpy as np

for _p in ("/opt/trn_rl_repo", "/root/.axon_site/_ro/trn_rl_repo"):
    if _p not in sys.path:
        sys.path.append(_p)

import concourse.bass as bass
import concourse.tile as tile
from concourse import bacc, mybir
from concourse.bass_utils import run_bass_kernel_spmd

B, C1, C2 = 65536, 100, 1000
NCORES = 8
BC = B // NCORES            # rows per core (8192)
P = 128                     # partitions
KS = 4                      # sub-rows per partition per tile
NT = BC // (P * KS)         # tiles per core (16)
NSEG = BC // P              # per-row segments per core (64) == NT*KS
NQ = 4                      # one-hot quarters (DVE op granularity)
JQ = NSEG // NQ             # segments per quarter (16)
F32 = mybir.dt.float32
F32R = mybir.dt.float32r
X = mybir.AxisListType.X
ALU = mybir.AluOpType
ACTF = mybir.ActivationFunctionType

# PSUM matmul chunking of the C2 free dim (PSUM bank holds 512 f32).
CHUNKS = [(0, 512), (512, C2)]
GA_LAST = NT - 4            # G splits: tiles 0..12 (folded early) / 13..15
# Exp row-sums fused on the ACT accumulator (mid-stream, where ACT idles
# waiting on DMA anyway); the rest reduce on the DVE.  Tail tiles stay on
# the DVE so the last segments' sums come off the parallel pipe; seg 63
# uses the ACT accumulator so the DVE is free for the G_b fold.
ACT_ACCUM_SEGS = frozenset(s for s in range(4, 56, 3)) | {NSEG - 1}


def _build_program(general: bool) -> bass.Bass:
    nc = bacc.Bacc("TRN2", target_bir_lowering=False, debug=False,
                   num_devices=NCORES)
    p1 = nc.dram_tensor("p1", [BC, C1], F32, kind="ExternalInput").ap()
    p2 = nc.dram_tensor("p2", [BC, C2], F32, kind="ExternalInput").ap()
    tbl = nc.dram_tensor("tbl", [C1, C2], F32, kind="ExternalInput").ap()
    sbc = None
    if general:
        # table row-sums broadcast over partitions (host constant)
        sbc = nc.dram_tensor("sbc", [P, C1], F32, kind="ExternalInput").ap()
    se_out = nc.dram_tensor("se", [P, NSEG], F32, kind="ExternalOutput").ap()
    rd_out = nc.dram_tensor("rd", [C1, 2], F32, kind="ExternalOutput").ap()
    sel_out = None
    if general:
        sel_out = nc.dram_tensor("sel", [P, NSEG], F32,
                                 kind="ExternalOutput").ap()

    with tile.TileContext(nc) as tc:
        with ExitStack() as ctx:
            _kernel_body(ctx, tc, p1, p2, tbl, sbc, se_out, rd_out, sel_out,
                         general)
    nc.compile()
    return nc


def _kernel_body(ctx: ExitStack, tc, p1, p2, tbl, sbc, se_out, rd_out,
                 sel_out, general):
    nc = tc.nc
    consts = ctx.enter_context(tc.tile_pool(name="consts", bufs=1))
    p2pool = ctx.enter_context(tc.tile_pool(name="p2", bufs=5))
    expp = ctx.enter_context(tc.tile_pool(name="expp", bufs=6))
    psum = ctx.enter_context(tc.tile_pool(name="psum", bufs=1, space="PSUM"))

    # row (p*64 + i*4 + k)  <->  tile i, partition p, sub-row k
    p2t = p2.rearrange("(p i k) c -> i p (k c)", p=P, i=NT, k=KS)
    # pred1 halves: per-partition 12.8KB contiguous runs
    p1h = p1.rearrange("(p h j) c -> h p (j c)", p=P, h=2, j=NSEG // 2)

    p1big = consts.tile([P, NSEG * C1], F32)
    oh_all = consts.tile([P, NSEG * C1], F32R)
    tbl_sb = consts.tile([C1, C2], F32)
    rmax = consts.tile([P, NSEG], F32)
    se_all = consts.tile([P, NSEG], F32)
    rowdots = consts.tile([C1, 2], F32)

    if general:
        sbc_sb = consts.tile([P, C1], F32)
        ss_scratch = consts.tile([P, JQ * C1], F32)
        ss3 = ss_scratch[:].rearrange("p (j c) -> p j c", j=JQ)
        sbc3 = sbc_sb[:].unsqueeze(1).broadcast_to((P, JQ, C1))
        sel_s_all = consts.tile([P, NSEG], F32)

    G_a = psum.tile([C1, C2], F32)         # onehot^T @ pred2, tiles 0..12
    G_b = psum.tile([C1, C2], F32)         # tiles 13..15

    p1big3 = p1big[:].rearrange("p (j c) -> p j c", j=NSEG)
    oh3 = oh_all[:].rearrange("p (j c) -> p j c", j=NSEG)

    def onehot_quarter(q):
        js = slice(q * JQ, (q + 1) * JQ)
        nc.vector.reduce_max(rmax[:, js], p1big3[:, js, :], axis=X)
        rmx3 = rmax[:, js].unsqueeze(2).broadcast_to((P, JQ, C1))
        nc.vector.tensor_tensor(oh3[:, js, :], p1big3[:, js, :], rmx3,
                                op=ALU.is_ge)
        if general:
            nc.vector.tensor_tensor(ss3[:], oh3[:, js, :].bitcast(F32), sbc3,
                                    op=ALU.mult)
            nc.vector.reduce_sum(sel_s_all[:, js], ss3[:], axis=X)

    t2_tiles = []

    def load_tile(i):
        t2 = p2pool.tile([P, KS * C2], F32R, tag="p2")
        if i == 0 or i == NT - 1:
            # k-split: pipeline head starts early / tail lands continuously
            for k in range(KS):
                nc.sync.dma_start(t2[:, bass.ts(k, C2)],
                                  p2t[i][:, bass.ts(k, C2)].bitcast(F32R))
        elif i >= NT - 3:
            # pair-split taper for the tiles feeding the serialized ACT tail
            for k in range(0, KS, 2):
                nc.sync.dma_start(t2[:, k * C2:(k + 2) * C2],
                                  p2t[i][:, k * C2:(k + 2) * C2].bitcast(F32R))
        else:
            nc.sync.dma_start(t2[:], p2t[i].bitcast(F32R))
        t2_tiles.append(t2)

    def consume_tile(i):
        t2 = t2_tiles[i]
        for k in range(KS):
            seg = i * KS + k
            se_col = se_all[:, seg:seg + 1]
            et = expp.tile([P, C2], F32, tag="exp1")
            if seg in ACT_ACCUM_SEGS:
                nc.scalar.activation(et[:], t2[:, bass.ts(k, C2)].bitcast(F32),
                                     ACTF.Exp, accum_out=se_col)
            else:
                nc.scalar.activation(et[:], t2[:, bass.ts(k, C2)].bitcast(F32),
                                     ACTF.Exp)
                nc.vector.reduce_sum(se_col, et[:], axis=X)
        G = G_a if i <= GA_LAST else G_b
        for k in range(KS):
            seg = i * KS + k
            for lo, hi in CHUNKS:
                nc.tensor.matmul(G[:, lo:hi], oh_all[:, bass.ts(seg, C1)],
                                 t2[:, k * C2 + lo:k * C2 + hi],
                                 start=(k == 0 and i in (0, GA_LAST + 1)),
                                 stop=(k == KS - 1 and i in (GA_LAST, NT - 1)))

    # pred1 + table ride the SWDGE queue in parallel with the sync-ring
    # pred2 stream: the two descriptor paths overlap, keeping HBM saturated
    # without serializing pred2 tiles behind pred1 in one FIFO.
    nc.gpsimd.dma_start(p1big[:, 0:NSEG * C1 // 2], p1h[0])
    nc.gpsimd.dma_start(p1big[:, NSEG * C1 // 2:], p1h[1])
    nc.gpsimd.dma_start(tbl_sb[:], tbl[:, :])
    if general:
        nc.gpsimd.dma_start(sbc_sb[:], sbc[:, :])
    for i in range(NT):
        load_tile(i)

    gt_scratch = consts.tile([C1, C2], F32)
    for i in range(NT):
        if i % 4 == 0:
            onehot_quarter(i // 4)
        consume_tile(i)
        if i == GA_LAST + 1:
            # G_a is complete; fold it against the table mid-stream (fused
            # multiply + negated row-reduce on the DVE, fully hidden) and
            # ship the result while pred2 still streams.
            nc.vector.tensor_mul(gt_scratch[:], G_a[:], tbl_sb[:])
            nc.vector.tensor_reduce(rowdots[:, 0:1], gt_scratch[:], axis=X,
                                    op=ALU.add, negate=True)

    # Tail: G_b fold on the DVE, then the two small result DMAs.
    nc.vector.tensor_mul(gt_scratch[:], G_b[:], tbl_sb[:])
    nc.vector.tensor_reduce(rowdots[:, 1:2], gt_scratch[:], axis=X,
                            op=ALU.add, negate=True)
    nc.sync.dma_start(se_out[:, :], se_all[:])
    if general:
        nc.sync.dma_start(sel_out[:, :], sel_s_all[:])
    nc.sync.dma_start(rd_out[:, :], rowdots[:])


_PROGRAM_CACHE: dict = {}


def _program(general: bool = False) -> bass.Bass:
    key = "general" if general else "fast"
    if key not in _PROGRAM_CACHE:
        _PROGRAM_CACHE[key] = _build_program(general)
    return _PROGRAM_CACHE[key]


def _row_sums(table):
    return np.asarray(table, dtype=np.float32).sum(axis=1, dtype=np.float32)


def _is_uniform_s(s):
    return bool(np.all(np.abs(s - s[0]) <= 1e-6 * max(1.0, abs(float(s[0])))))


def _in_maps(pred1_logits, pred2_logits, table, general: bool = False):
    p1 = np.ascontiguousarray(pred1_logits, dtype=np.float32)
    p2 = np.ascontiguousarray(pred2_logits, dtype=np.float32)
    tbl = np.ascontiguousarray(table, dtype=np.float32)
    maps = []
    for k in range(NCORES):
        m = {
            "p1": np.ascontiguousarray(p1[k * BC:(k + 1) * BC]),
            "p2": np.ascontiguousarray(p2[k * BC:(k + 1) * BC]),
            "tbl": tbl,
        }
        if general:
            s = _row_sums(tbl)
            m["sbc"] = np.ascontiguousarray(np.tile(s, (P, 1)))
        maps.append(m)
    return maps


def _combine(result, s0, general):
    lse = np.log(np.asarray(result["se"], dtype=np.float64))
    if general:
        lse = lse * np.asarray(result["sel"], dtype=np.float64)
        lse_term = lse.sum()
    else:
        lse_term = s0 * lse.sum()
    return lse_term + np.asarray(result["rd"], dtype=np.float64).sum()


def run_on_device(pred1_logits, pred2_logits, table, **spmd_kwargs):
    """Compile/run the SPMD program on cores 0-7; returns (loss, results)."""
    s = _row_sums(table)
    general = not _is_uniform_s(s)
    nc = _program(general)
    res = run_bass_kernel_spmd(
        nc, _in_maps(pred1_logits, pred2_logits, table, general),
        core_ids=list(range(NCORES)), **spmd_kwargs)
    s0 = np.float64(s[0])
    partials = [_combine(r, s0, general) for r in res.results]
    loss = np.float32(np.sum(partials, dtype=np.float64) / B)
    return np.asarray(loss), res


def kernel(pred1_logits, pred2_logits, table):
    loss, _ = run_on_device(pred1_logits, pred2_logits, table)
    return loss


# revision 23
# speedup vs baseline: 1.1312x; 1.0292x over previous
"""Trainium2 Bass kernel for the consistency-loss problem.

loss = -mean_b( table[argmax_c pred1[b,c]] . log_softmax(pred2[b]) )

Algebra used on-device (per batch row b, with c* = argmax of pred1 row):
    loss_b = lse_b * s[c*] - table[c*] . pred2[b]
where lse_b = log(sum_j exp(pred2[b,j])) and s[c] = sum_j table[c,j].

The expensive dot term summed over the batch factorizes through a matmul in
the natural (row-major) layout:
    sum_b table[c*_b] . pred2[b] = sum_{c,j} table[c,j] * G[c,j],
    G = onehot(c*)^T @ pred2          (contraction over batch rows)
so the PE accumulates G in PSUM across row-tiles (f32r) with no transposes
of the big [B, 1000] tensor.

The kernel is HBM-bound (~358 GB/s per core): 36.4 MB of inputs stream in
~102 us, so the design minimizes everything off the stream:
 - pred1 + table ride the SWDGE (gpsimd) queue starting ~2.5 us, before the
   sync-ring HWDGE descriptors are even generated; pred2 owns the sync ring.
 - row layout r = p*64 + i*4 + k gives 16 KB contiguous runs per pred2 tile
   and 12.8 KB runs for the two pred1 half-loads.
 - per-row exp sums: ACT Exp (some with fused accumulation), DVE row-sum
   reduces; the one-hot compare runs as four broadcasted DVE ops.
 - no device-side log: the [128, 64] exp row-sums are shipped to the host
   (32 KB), which applies log in f64 — this removes the natural_log ACT
   table switch (~2.7 us) and the final reduction chain from the tail.
 - G folds against the table on the DVE: tiles 0..12 fold mid-stream
   (hidden), tiles 13..15 fold in the ~2.4 us tail.

The harness table is row-stochastic (uniform row sums), so the fast program
skips the per-row s[c*] gather; the host scales sum(log se) by s0.  A
general program (per-row s via the one-hot) remains as a fallback.

Sharding: data-parallel over B across 8 NeuronCores; the [100,1000] table
is replicated; the host combines per-core partial outputs.
"""

import sys
from contextlib import ExitStack

import numpy as np

for _p in ("/opt/trn_rl_repo", "/root/.axon_site/_ro/trn_rl_repo"):
    if _p not in sys.path:
        sys.path.append(_p)

import concourse.bass as bass
import concourse.tile as tile
from concourse import bacc, mybir
from concourse.bass_utils import run_bass_kernel_spmd

B, C1, C2 = 65536, 100, 1000
NCORES = 8
BC = B // NCORES            # rows per core (8192)
P = 128                     # partitions
KS = 4                      # sub-rows per partition per tile
NT = BC // (P * KS)         # tiles per core (16)
NSEG = BC // P              # per-row segments per core (64) == NT*KS
NQ = 4                      # one-hot quarters (DVE op granularity)
JQ = NSEG // NQ             # segments per quarter (16)
F32 = mybir.dt.float32
F32R = mybir.dt.float32r
X = mybir.AxisListType.X
ALU = mybir.AluOpType
ACTF = mybir.ActivationFunctionType

# PSUM matmul chunking of the C2 free dim (PSUM bank holds 512 f32).
CHUNKS = [(0, 512), (512, C2)]
GA_LAST = NT - 4            # G splits: tiles 0..12 (folded early) / 13..15
SINGLE_RING = True          # pred1/table on the sync ring vs SWDGE
# Exp row-sums fused on the ACT accumulator (mid-stream, where ACT idles
# waiting on DMA anyway); the rest reduce on the DVE.  Tail tiles stay on
# the DVE so the last segments' sums come off the parallel pipe; seg 63
# uses the ACT accumulator so the DVE is free for the G_b fold.
ACT_ACCUM_SEGS = frozenset(s for s in range(4, 56, 3)) | {NSEG - 1}


def _build_program(general: bool) -> bass.Bass:
    nc = bacc.Bacc("TRN2", target_bir_lowering=False, debug=False,
                   num_devices=NCORES)
    p1 = nc.dram_tensor("p1", [BC, C1], F32, kind="ExternalInput").ap()
    p2 = nc.dram_tensor("p2", [BC, C2], F32, kind="ExternalInput").ap()
    tbl = nc.dram_tensor("tbl", [C1, C2], F32, kind="ExternalInput").ap()
    sbc = None
    if general:
        # table row-sums broadcast over partitions (host constant)
        sbc = nc.dram_tensor("sbc", [P, C1], F32, kind="ExternalInput").ap()
    se_out = nc.dram_tensor("se", [P, NSEG], F32, kind="ExternalOutput").ap()
    rd_out = nc.dram_tensor("rd", [C1, 2], F32, kind="ExternalOutput").ap()
    sel_out = None
    if general:
        sel_out = nc.dram_tensor("sel", [P, NSEG], F32,
                                 kind="ExternalOutput").ap()

    with tile.TileContext(nc) as tc:
        with ExitStack() as ctx:
            _kernel_body(ctx, tc, p1, p2, tbl, sbc, se_out, rd_out, sel_out,
                         general)
    nc.compile()
    return nc


def _kernel_body(ctx: ExitStack, tc, p1, p2, tbl, sbc, se_out, rd_out,
                 sel_out, general):
    nc = tc.nc
    consts = ctx.enter_context(tc.tile_pool(name="consts", bufs=1))
    p2pool = ctx.enter_context(tc.tile_pool(name="p2", bufs=5))
    expp = ctx.enter_context(tc.tile_pool(name="expp", bufs=6))
    psum = ctx.enter_context(tc.tile_pool(name="psum", bufs=1, space="PSUM"))

    # row (p*64 + i*4 + k)  <->  tile i, partition p, sub-row k
    p2t = p2.rearrange("(p i k) c -> i p (k c)", p=P, i=NT, k=KS)
    # pred1 halves: per-partition 12.8KB contiguous runs
    p1h = p1.rearrange("(p h j) c -> h p (j c)", p=P, h=2, j=NSEG // 2)

    p1big = consts.tile([P, NSEG * C1], F32)
    oh_all = consts.tile([P, NSEG * C1], F32R)
    tbl_sb = consts.tile([C1, C2], F32)
    rmax = consts.tile([P, NSEG], F32)
    se_all = consts.tile([P, NSEG], F32)
    rowdots = consts.tile([C1, 2], F32)

    if general:
        sbc_sb = consts.tile([P, C1], F32)
        ss_scratch = consts.tile([P, JQ * C1], F32)
        ss3 = ss_scratch[:].rearrange("p (j c) -> p j c", j=JQ)
        sbc3 = sbc_sb[:].unsqueeze(1).broadcast_to((P, JQ, C1))
        sel_s_all = consts.tile([P, NSEG], F32)

    G_a = psum.tile([C1, C2], F32)         # onehot^T @ pred2, tiles 0..12
    G_b = psum.tile([C1, C2], F32)         # tiles 13..15

    p1big3 = p1big[:].rearrange("p (j c) -> p j c", j=NSEG)
    oh3 = oh_all[:].rearrange("p (j c) -> p j c", j=NSEG)

    def onehot_quarter(q):
        js = slice(q * JQ, (q + 1) * JQ)
        nc.vector.reduce_max(rmax[:, js], p1big3[:, js, :], axis=X)
        rmx3 = rmax[:, js].unsqueeze(2).broadcast_to((P, JQ, C1))
        nc.vector.tensor_tensor(oh3[:, js, :], p1big3[:, js, :], rmx3,
                                op=ALU.is_ge)
        if general:
            nc.vector.tensor_tensor(ss3[:], oh3[:, js, :].bitcast(F32), sbc3,
                                    op=ALU.mult)
            nc.vector.reduce_sum(sel_s_all[:, js], ss3[:], axis=X)

    t2_tiles = []

    def load_tile(i):
        t2 = p2pool.tile([P, KS * C2], F32R, tag="p2")
        if i == 0 or i == NT - 1:
            # k-split: pipeline head starts early / tail lands continuously
            for k in range(KS):
                nc.sync.dma_start(t2[:, bass.ts(k, C2)],
                                  p2t[i][:, bass.ts(k, C2)].bitcast(F32R))
        elif i >= NT - 3:
            # pair-split taper for the tiles feeding the serialized ACT tail
            for k in range(0, KS, 2):
                nc.sync.dma_start(t2[:, k * C2:(k + 2) * C2],
                                  p2t[i][:, k * C2:(k + 2) * C2].bitcast(F32R))
        else:
            nc.sync.dma_start(t2[:], p2t[i].bitcast(F32R))
        t2_tiles.append(t2)

    def consume_tile(i):
        t2 = t2_tiles[i]
        for k in range(KS):
            seg = i * KS + k
            se_col = se_all[:, seg:seg + 1]
            et = expp.tile([P, C2], F32, tag="exp1")
            if seg in ACT_ACCUM_SEGS:
                nc.scalar.activation(et[:], t2[:, bass.ts(k, C2)].bitcast(F32),
                                     ACTF.Exp, accum_out=se_col)
            else:
                nc.scalar.activation(et[:], t2[:, bass.ts(k, C2)].bitcast(F32),
                                     ACTF.Exp)
                nc.vector.reduce_sum(se_col, et[:], axis=X)
        G = G_a if i <= GA_LAST else G_b
        for k in range(KS):
            seg = i * KS + k
            for lo, hi in CHUNKS:
                nc.tensor.matmul(G[:, lo:hi], oh_all[:, bass.ts(seg, C1)],
                                 t2[:, k * C2 + lo:k * C2 + hi],
                                 start=(k == 0 and i in (0, GA_LAST + 1)),
                                 stop=(k == KS - 1 and i in (GA_LAST, NT - 1)))

    if SINGLE_RING:
        nc.sync.dma_start(p1big[:, 0:NSEG * C1 // 2], p1h[0])
        load_tile(0)
        load_tile(1)
        nc.sync.dma_start(p1big[:, NSEG * C1 // 2:], p1h[1])
        load_tile(2)
        nc.sync.dma_start(tbl_sb[:], tbl[:, :])
        if general:
            nc.sync.dma_start(sbc_sb[:], sbc[:, :])
        for i in range(3, NT):
            load_tile(i)
    else:
        # pred1 + table ride the SWDGE queue in parallel with the sync-ring
        # pred2 stream: the two descriptor paths overlap, keeping HBM
        # saturated without serializing pred2 behind pred1 in one FIFO.
        nc.gpsimd.dma_start(p1big[:, 0:NSEG * C1 // 2], p1h[0])
        nc.gpsimd.dma_start(p1big[:, NSEG * C1 // 2:], p1h[1])
        nc.gpsimd.dma_start(tbl_sb[:], tbl[:, :])
        if general:
            nc.gpsimd.dma_start(sbc_sb[:], sbc[:, :])
        for i in range(NT):
            load_tile(i)

    gt_scratch = consts.tile([C1, C2], F32)
    for i in range(NT):
        if i % 4 == 0:
            onehot_quarter(i // 4)
        consume_tile(i)
        if i == GA_LAST + 1:
            # G_a is complete; fold it against the table mid-stream (fused
            # multiply + negated row-reduce on the DVE, fully hidden) and
            # ship the result while pred2 still streams.
            nc.vector.tensor_mul(gt_scratch[:], G_a[:], tbl_sb[:])
            nc.vector.tensor_reduce(rowdots[:, 0:1], gt_scratch[:], axis=X,
                                    op=ALU.add, negate=True)

    # Tail: G_b fold on the DVE, then the two small result DMAs.
    nc.vector.tensor_mul(gt_scratch[:], G_b[:], tbl_sb[:])
    nc.vector.tensor_reduce(rowdots[:, 1:2], gt_scratch[:], axis=X,
                            op=ALU.add, negate=True)
    nc.sync.dma_start(se_out[:, :], se_all[:])
    if general:
        nc.sync.dma_start(sel_out[:, :], sel_s_all[:])
    nc.sync.dma_start(rd_out[:, :], rowdots[:])


_PROGRAM_CACHE: dict = {}


def _program(general: bool = False) -> bass.Bass:
    key = "general" if general else "fast"
    if key not in _PROGRAM_CACHE:
        _PROGRAM_CACHE[key] = _build_program(general)
    return _PROGRAM_CACHE[key]


def _row_sums(table):
    return np.asarray(table, dtype=np.float32).sum(axis=1, dtype=np.float32)


def _is_uniform_s(s):
    return bool(np.all(np.abs(s - s[0]) <= 1e-6 * max(1.0, abs(float(s[0])))))


def _in_maps(pred1_logits, pred2_logits, table, general: bool = False):
    p1 = np.ascontiguousarray(pred1_logits, dtype=np.float32)
    p2 = np.ascontiguousarray(pred2_logits, dtype=np.float32)
    tbl = np.ascontiguousarray(table, dtype=np.float32)
    maps = []
    for k in range(NCORES):
        m = {
            "p1": np.ascontiguousarray(p1[k * BC:(k + 1) * BC]),
            "p2": np.ascontiguousarray(p2[k * BC:(k + 1) * BC]),
            "tbl": tbl,
        }
        if general:
            s = _row_sums(tbl)
            m["sbc"] = np.ascontiguousarray(np.tile(s, (P, 1)))
        maps.append(m)
    return maps


def _combine(result, s0, general):
    lse = np.log(np.asarray(result["se"], dtype=np.float64))
    if general:
        lse = lse * np.asarray(result["sel"], dtype=np.float64)
        lse_term = lse.sum()
    else:
        lse_term = s0 * lse.sum()
    return lse_term + np.asarray(result["rd"], dtype=np.float64).sum()


def run_on_device(pred1_logits, pred2_logits, table, **spmd_kwargs):
    """Compile/run the SPMD program on cores 0-7; returns (loss, results)."""
    s = _row_sums(table)
    general = not _is_uniform_s(s)
    nc = _program(general)
    res = run_bass_kernel_spmd(
        nc, _in_maps(pred1_logits, pred2_logits, table, general),
        core_ids=list(range(NCORES)), **spmd_kwargs)
    s0 = np.float64(s[0])
    partials = [_combine(r, s0, general) for r in res.results]
    loss = np.float32(np.sum(partials, dtype=np.float64) / B)
    return np.asarray(loss), res


def kernel(pred1_logits, pred2_logits, table):
    loss, _ = run_on_device(pred1_logits, pred2_logits, table)
    return loss


# revision 24
# speedup vs baseline: 1.1418x; 1.0094x over previous
"""Trainium2 Bass kernel for the consistency-loss problem.

loss = -mean_b( table[argmax_c pred1[b,c]] . log_softmax(pred2[b]) )

Algebra used on-device (per batch row b, with c* = argmax of pred1 row):
    loss_b = lse_b * s[c*] - table[c*] . pred2[b]
where lse_b = log(sum_j exp(pred2[b,j])) and s[c] = sum_j table[c,j].

The expensive dot term summed over the batch factorizes through a matmul in
the natural (row-major) layout:
    sum_b table[c*_b] . pred2[b] = sum_{c,j} table[c,j] * G[c,j],
    G = onehot(c*)^T @ pred2          (contraction over batch rows)
so the PE accumulates G in PSUM across row-tiles (f32r) with no transposes
of the big [B, 1000] tensor.

The kernel is HBM-bound (~358 GB/s per core): 36.4 MB of inputs stream in
~102 us, so the design minimizes everything off the stream:
 - pred1 + table ride the SWDGE (gpsimd) queue starting ~2.5 us, before the
   sync-ring HWDGE descriptors are even generated; pred2 owns the sync ring.
 - row layout r = p*64 + i*4 + k gives 16 KB contiguous runs per pred2 tile
   and 12.8 KB runs for the two pred1 half-loads.
 - per-row exp sums: ACT Exp (some with fused accumulation), DVE row-sum
   reduces; the one-hot compare runs as four broadcasted DVE ops.
 - no device-side log: the [128, 64] exp row-sums are shipped to the host
   (32 KB), which applies log in f64 — this removes the natural_log ACT
   table switch (~2.7 us) and the final reduction chain from the tail.
 - G folds against the table on the DVE: tiles 0..12 fold mid-stream
   (hidden), tiles 13..15 fold in the ~2.4 us tail.

The harness table is row-stochastic (uniform row sums), so the fast program
skips the per-row s[c*] gather; the host scales sum(log se) by s0.  A
general program (per-row s via the one-hot) remains as a fallback.

Sharding: data-parallel over B across 8 NeuronCores; the [100,1000] table
is replicated; the host combines per-core partial outputs.
"""

import sys
from contextlib import ExitStack

import numpy as np

for _p in ("/opt/trn_rl_repo", "/root/.axon_site/_ro/trn_rl_repo"):
    if _p not in sys.path:
        sys.path.append(_p)

import concourse.bass as bass
import concourse.tile as tile
from concourse import bacc, mybir
from concourse.bass_utils import run_bass_kernel_spmd

B, C1, C2 = 65536, 100, 1000
NCORES = 8
BC = B // NCORES            # rows per core (8192)
P = 128                     # partitions
KS = 4                      # sub-rows per partition per tile
NT = BC // (P * KS)         # tiles per core (16)
NSEG = BC // P              # per-row segments per core (64) == NT*KS
NQ = 4                      # one-hot quarters (DVE op granularity)
JQ = NSEG // NQ             # segments per quarter (16)
F32 = mybir.dt.float32
F32R = mybir.dt.float32r
X = mybir.AxisListType.X
ALU = mybir.AluOpType
ACTF = mybir.ActivationFunctionType

# PSUM matmul chunking of the C2 free dim (PSUM bank holds 512 f32).
CHUNKS = [(0, 512), (512, C2)]
GA_LAST = NT - 4            # G splits: tiles 0..12 (folded early) / 13..15
SINGLE_RING = True          # pred1/table on the sync ring vs SWDGE
# Exp row-sums fused on the ACT accumulator (mid-stream, where ACT idles
# waiting on DMA anyway); the rest reduce on the DVE.  Tail tiles stay on
# the DVE so the last segments' sums come off the parallel pipe; seg 63
# uses the ACT accumulator so the DVE is free for the G_b fold.
ACT_ACCUM_SEGS = frozenset(s for s in range(4, 56, 3)) | {NSEG - 2, NSEG - 1}


def _build_program(general: bool) -> bass.Bass:
    nc = bacc.Bacc("TRN2", target_bir_lowering=False, debug=False,
                   num_devices=NCORES)
    p1 = nc.dram_tensor("p1", [BC, C1], F32, kind="ExternalInput").ap()
    p2 = nc.dram_tensor("p2", [BC, C2], F32, kind="ExternalInput").ap()
    tbl = nc.dram_tensor("tbl", [C1, C2], F32, kind="ExternalInput").ap()
    sbc = None
    if general:
        # table row-sums broadcast over partitions (host constant)
        sbc = nc.dram_tensor("sbc", [P, C1], F32, kind="ExternalInput").ap()
    se_out = nc.dram_tensor("se", [P, NSEG], F32, kind="ExternalOutput").ap()
    rd_out = nc.dram_tensor("rd", [C1, 2], F32, kind="ExternalOutput").ap()
    sel_out = None
    if general:
        sel_out = nc.dram_tensor("sel", [P, NSEG], F32,
                                 kind="ExternalOutput").ap()

    with tile.TileContext(nc) as tc:
        with ExitStack() as ctx:
            _kernel_body(ctx, tc, p1, p2, tbl, sbc, se_out, rd_out, sel_out,
                         general)
    nc.compile()
    return nc


def _kernel_body(ctx: ExitStack, tc, p1, p2, tbl, sbc, se_out, rd_out,
                 sel_out, general):
    nc = tc.nc
    consts = ctx.enter_context(tc.tile_pool(name="consts", bufs=1))
    p2pool = ctx.enter_context(tc.tile_pool(name="p2", bufs=5))
    expp = ctx.enter_context(tc.tile_pool(name="expp", bufs=6))
    psum = ctx.enter_context(tc.tile_pool(name="psum", bufs=1, space="PSUM"))

    # row (p*64 + i*4 + k)  <->  tile i, partition p, sub-row k
    p2t = p2.rearrange("(p i k) c -> i p (k c)", p=P, i=NT, k=KS)
    # pred1 halves: per-partition 12.8KB contiguous runs
    p1h = p1.rearrange("(p h j) c -> h p (j c)", p=P, h=2, j=NSEG // 2)

    p1big = consts.tile([P, NSEG * C1], F32)
    oh_all = consts.tile([P, NSEG * C1], F32R)
    tbl_sb = consts.tile([C1, C2], F32)
    rmax = consts.tile([P, NSEG], F32)
    se_all = consts.tile([P, NSEG], F32)
    rowdots = consts.tile([C1, 2], F32)

    if general:
        sbc_sb = consts.tile([P, C1], F32)
        ss_scratch = consts.tile([P, JQ * C1], F32)
        ss3 = ss_scratch[:].rearrange("p (j c) -> p j c", j=JQ)
        sbc3 = sbc_sb[:].unsqueeze(1).broadcast_to((P, JQ, C1))
        sel_s_all = consts.tile([P, NSEG], F32)

    G_a = psum.tile([C1, C2], F32)         # onehot^T @ pred2, tiles 0..12
    G_b = psum.tile([C1, C2], F32)         # tiles 13..15

    p1big3 = p1big[:].rearrange("p (j c) -> p j c", j=NSEG)
    oh3 = oh_all[:].rearrange("p (j c) -> p j c", j=NSEG)

    def onehot_quarter(q):
        js = slice(q * JQ, (q + 1) * JQ)
        nc.vector.reduce_max(rmax[:, js], p1big3[:, js, :], axis=X)
        rmx3 = rmax[:, js].unsqueeze(2).broadcast_to((P, JQ, C1))
        nc.vector.tensor_tensor(oh3[:, js, :], p1big3[:, js, :], rmx3,
                                op=ALU.is_ge)
        if general:
            nc.vector.tensor_tensor(ss3[:], oh3[:, js, :].bitcast(F32), sbc3,
                                    op=ALU.mult)
            nc.vector.reduce_sum(sel_s_all[:, js], ss3[:], axis=X)

    t2_tiles = []

    def load_tile(i):
        t2 = p2pool.tile([P, KS * C2], F32R, tag="p2")
        if i == 0 or i == NT - 1:
            # k-split: pipeline head starts early / tail lands continuously
            for k in range(KS):
                nc.sync.dma_start(t2[:, bass.ts(k, C2)],
                                  p2t[i][:, bass.ts(k, C2)].bitcast(F32R))
        elif i >= NT - 3:
            # pair-split taper for the tiles feeding the serialized ACT tail
            for k in range(0, KS, 2):
                nc.sync.dma_start(t2[:, k * C2:(k + 2) * C2],
                                  p2t[i][:, k * C2:(k + 2) * C2].bitcast(F32R))
        else:
            nc.sync.dma_start(t2[:], p2t[i].bitcast(F32R))
        t2_tiles.append(t2)

    def consume_tile(i):
        t2 = t2_tiles[i]
        for k in range(KS):
            seg = i * KS + k
            se_col = se_all[:, seg:seg + 1]
            et = expp.tile([P, C2], F32, tag="exp1")
            if seg in ACT_ACCUM_SEGS:
                nc.scalar.activation(et[:], t2[:, bass.ts(k, C2)].bitcast(F32),
                                     ACTF.Exp, accum_out=se_col)
            else:
                nc.scalar.activation(et[:], t2[:, bass.ts(k, C2)].bitcast(F32),
                                     ACTF.Exp)
                nc.vector.reduce_sum(se_col, et[:], axis=X)
        G = G_a if i <= GA_LAST else G_b
        for k in range(KS):
            seg = i * KS + k
            for lo, hi in CHUNKS:
                nc.tensor.matmul(G[:, lo:hi], oh_all[:, bass.ts(seg, C1)],
                                 t2[:, k * C2 + lo:k * C2 + hi],
                                 start=(k == 0 and i in (0, GA_LAST + 1)),
                                 stop=(k == KS - 1 and i in (GA_LAST, NT - 1)))

    if SINGLE_RING:
        nc.sync.dma_start(p1big[:, 0:NSEG * C1 // 2], p1h[0])
        load_tile(0)
        load_tile(1)
        nc.sync.dma_start(p1big[:, NSEG * C1 // 2:], p1h[1])
        load_tile(2)
        nc.sync.dma_start(tbl_sb[:], tbl[:, :])
        if general:
            nc.sync.dma_start(sbc_sb[:], sbc[:, :])
        for i in range(3, NT):
            load_tile(i)
    else:
        # pred1 + table ride the SWDGE queue in parallel with the sync-ring
        # pred2 stream: the two descriptor paths overlap, keeping HBM
        # saturated without serializing pred2 behind pred1 in one FIFO.
        nc.gpsimd.dma_start(p1big[:, 0:NSEG * C1 // 2], p1h[0])
        nc.gpsimd.dma_start(p1big[:, NSEG * C1 // 2:], p1h[1])
        nc.gpsimd.dma_start(tbl_sb[:], tbl[:, :])
        if general:
            nc.gpsimd.dma_start(sbc_sb[:], sbc[:, :])
        for i in range(NT):
            load_tile(i)

    gt_scratch = consts.tile([C1, C2], F32)
    for i in range(NT):
        if i % 4 == 0:
            onehot_quarter(i // 4)
        consume_tile(i)
        if i == GA_LAST + 1:
            # G_a is complete; fold it against the table mid-stream (fused
            # multiply + negated row-reduce on the DVE, fully hidden) and
            # ship the result while pred2 still streams.
            nc.vector.tensor_mul(gt_scratch[:], G_a[:], tbl_sb[:])
            nc.vector.tensor_reduce(rowdots[:, 0:1], gt_scratch[:], axis=X,
                                    op=ALU.add, negate=True)

    # Tail: G_b fold on the DVE, then the two small result DMAs.
    nc.vector.tensor_mul(gt_scratch[:], G_b[:], tbl_sb[:])
    nc.vector.tensor_reduce(rowdots[:, 1:2], gt_scratch[:], axis=X,
                            op=ALU.add, negate=True)
    nc.sync.dma_start(se_out[:, :], se_all[:])
    if general:
        nc.sync.dma_start(sel_out[:, :], sel_s_all[:])
    nc.sync.dma_start(rd_out[:, :], rowdots[:])


_PROGRAM_CACHE: dict = {}


def _program(general: bool = False) -> bass.Bass:
    key = "general" if general else "fast"
    if key not in _PROGRAM_CACHE:
        _PROGRAM_CACHE[key] = _build_program(general)
    return _PROGRAM_CACHE[key]


def _row_sums(table):
    return np.asarray(table, dtype=np.float32).sum(axis=1, dtype=np.float32)


def _is_uniform_s(s):
    return bool(np.all(np.abs(s - s[0]) <= 1e-6 * max(1.0, abs(float(s[0])))))


def _in_maps(pred1_logits, pred2_logits, table, general: bool = False):
    p1 = np.ascontiguousarray(pred1_logits, dtype=np.float32)
    p2 = np.ascontiguousarray(pred2_logits, dtype=np.float32)
    tbl = np.ascontiguousarray(table, dtype=np.float32)
    maps = []
    for k in range(NCORES):
        m = {
            "p1": np.ascontiguousarray(p1[k * BC:(k + 1) * BC]),
            "p2": np.ascontiguousarray(p2[k * BC:(k + 1) * BC]),
            "tbl": tbl,
        }
        if general:
            s = _row_sums(tbl)
            m["sbc"] = np.ascontiguousarray(np.tile(s, (P, 1)))
        maps.append(m)
    return maps


def _combine(result, s0, general):
    lse = np.log(np.asarray(result["se"], dtype=np.float64))
    if general:
        lse = lse * np.asarray(result["sel"], dtype=np.float64)
        lse_term = lse.sum()
    else:
        lse_term = s0 * lse.sum()
    return lse_term + np.asarray(result["rd"], dtype=np.float64).sum()


def run_on_device(pred1_logits, pred2_logits, table, **spmd_kwargs):
    """Compile/run the SPMD program on cores 0-7; returns (loss, results)."""
    s = _row_sums(table)
    general = not _is_uniform_s(s)
    nc = _program(general)
    res = run_bass_kernel_spmd(
        nc, _in_maps(pred1_logits, pred2_logits, table, general),
        core_ids=list(range(NCORES)), **spmd_kwargs)
    s0 = np.float64(s[0])
    partials = [_combine(r, s0, general) for r in res.results]
    loss = np.float32(np.sum(partials, dtype=np.float64) / B)
    return np.asarray(loss), res


def kernel(pred1_logits, pred2_logits, table):
    loss, _ = run_on_device(pred1_logits, pred2_logits, table)
    return loss
